# revision 27
# baseline (speedup 1.0000x reference)
"""Trainium2 Bass kernel for nn_FeatureFusionModule.

Strategy: data-parallel over batch (16 images -> 8 cores x 2 images).
All activations feature-major [C, N] (C on partitions, N=H*W free), bf16
compute with fp32 PSUM. Linear-attention ctx accumulated on PE via
token-major kv; softmax normalization folded into the attention-output
copy (per-output-row reciprocal scale). LayerNorm gamma/beta folded into
the ChannelEmbed 1x1-conv weights. Depthwise 3x3 conv done as 9
diagonal-matmul accumulations with shifted access patterns. Both
BatchNorms resolved with ONE tiny AllReduce of 5 per-channel partial
sums; final output is the per-channel affine A*residual + B*e + C.
"""

import numpy as np

import concourse.bass as bass
import concourse.mybir as mybir
import concourse.tile as tile
from concourse import bacc
from concourse.bass_utils import run_bass_kernel_spmd
from concourse.masks import make_identity

F32 = mybir.dt.float32
BF16 = mybir.dt.bfloat16
AF = mybir.ActivationFunctionType
OP = mybir.AluOpType

N_CORES = 8
IMG = 2                  # images per core
C = 256
N = 4096                 # H*W
NCH = 8                  # feature-major n chunks
NCK = 512
EPS = 1e-5
SCALE = 32 ** -0.5       # head-dim^-0.5
NGLOB = float(16 * 4096)  # BatchNorm reduction count (full batch)


def build_nc(collective=True):
    nc = bacc.Bacc(None, target_bir_lowering=False, num_devices=N_CORES if collective else 1)

    def din(name, shape):
        return nc.dram_tensor(name, shape, F32, kind="ExternalInput")

    x1 = din("x1", [IMG, C, N])
    x2 = din("x2", [IMG, C, N])
    xd = [x1, x2]
    cp_w = [din("cp1_w", [512, 256]), din("cp2_w", [512, 256])]
    cp_b = [din("cp1_b", [512]), din("cp2_b", [512])]
    kv_w = [din("kv1_w", [512, 256]), din("kv2_w", [512, 256])]
    ep_w = [din("ep1_w", [256, 512]), din("ep2_w", [256, 512])]
    ep_b = [din("ep1_b", [256]), din("ep2_b", [256])]
    ln_g = [din("ln1_g", [256]), din("ln1_b", [256]),
            din("ln2_g", [256]), din("ln2_b", [256])]
    res_w = din("res_w", [256, 512])
    ce1_w = din("ce1_w", [256, 512])
    ce1_b = din("ce1_b", [256])
    dw_w = din("dw_w", [256, 9])
    dw_b = din("dw_b", [256])
    ce3_w = din("ce3_w", [256, 256])
    ce3_b = din("ce3_b", [256])
    bn = {k: din(k, [256]) for k in ("bn1_g", "bn1_b", "bn2_g", "bn2_b")}
    y = nc.dram_tensor("y", [IMG, C, N], F32, kind="ExternalOutput")

    with tile.TileContext(nc) as tc:
        wpool = tc.tile_pool(name="wpool", bufs=1)
        wp = wpool.__enter__()

        # ---------- persistent weight tiles ----------
        I_f32 = wp.tile([128, 128], F32, tag="ident", name="ident")
        make_identity(nc, I_f32[:])
        ones_col = wp.tile([128, 1], BF16, tag="ones_col", name="ones_col")
        nc.vector.memset(ones_col[:], 1.0)
        eps_col = wp.tile([128, 1], F32, tag="eps_col", name="eps_col")
        nc.vector.memset(eps_col[:], EPS)

        cpwT = [[wp.tile([128, 512], BF16, tag=f"cpwT{p}{k}", name=f"cpwT{p}{k}") for k in range(2)] for p in range(2)]
        kvwT = [[wp.tile([128, 512], BF16, tag=f"kvwT{p}{k}", name=f"kvwT{p}{k}") for k in range(2)] for p in range(2)]
        epwT = [[wp.tile([128, 256], BF16, tag=f"epwT{p}{k}", name=f"epwT{p}{k}") for k in range(4)] for p in range(2)]
        reswT = [wp.tile([128, 256], BF16, tag=f"reswT{k}", name=f"reswT{k}") for k in range(4)]
        ce1wT = [wp.tile([128, 256], BF16, tag=f"ce1wT{k}", name=f"ce1wT{k}") for k in range(4)]
        ce3wT = [wp.tile([128, 256], BF16, tag=f"ce3wT{k}", name=f"ce3wT{k}") for k in range(2)]

        def col(dram, c0, pool, tag, dt=F32):
            t = pool.tile([128, 1], dt, tag=tag, name=tag)
            nc.scalar.dma_start(out=t[:], in_=dram[c0:c0 + 128].rearrange("(p o) -> p o", o=1))
            return t

        resb = [wp.tile([128, 1], F32, tag=f"resb{k}", name=f"resb{k}") for k in range(2)]
        ce1b = [wp.tile([128, 1], F32, tag=f"ce1b{k}", name=f"ce1b{k}") for k in range(2)]
        glob = wp.tile([128, 10], F32, tag="glob", name="glob")
        stats_acc = wp.tile([128, 5, 2, 2 * NCH], F32, tag="stats_acc", name="stats_acc")

        # ---------- weight transposes (PE, fp32) ----------
        with tc.tile_pool(name="prep", bufs=4) as prep, \
             tc.tile_pool(name="prep_ps", bufs=2, space="PSUM") as prep_ps, \
             tc.tile_pool(name="prep_ps1", bufs=1, space="PSUM") as prep_ps1:

            qtoggle = [0]

            def transpose_into(wd, rchunks, ichunks, dest, scale_cols=None, bias_dest=None,
                               bias_extra=None):
                # wd: dram [R, Cin]; dest[ic] tiles [128, R]; optional per-in-row scale,
                # optional bias fold: bias_dest[rc] <- sum_in wT[in, out] * bcol[in] (+extra)
                for rc in range(rchunks):
                    st = prep.tile([128, ichunks * 128], F32, tag="wstage", name="wstage")
                    qtoggle[0] ^= 1
                    eng = nc.scalar if qtoggle[0] else nc.sync
                    eng.dma_start(out=st[:, 0:ichunks * 128], in_=wd[rc * 128:(rc + 1) * 128, :])
                    if bias_dest is not None:
                        bps = prep_ps1.tile([128, 1], F32, tag="bias_ps", name="bias_ps")
                    for ic in range(ichunks):
                        pst = prep_ps.tile([128, 128], F32, tag="ps_t", name="ps_t")
                        nc.tensor.matmul(pst[:], st[:, ic * 128:(ic + 1) * 128], I_f32[:],
                                         start=True, stop=True)
                        if scale_cols is not None:
                            nc.vector.tensor_scalar_mul(out=dest[ic][:, rc * 128:(rc + 1) * 128],
                                                        in0=pst[:], scalar1=scale_cols[ic][:, 0:1])
                        else:
                            nc.vector.tensor_copy(out=dest[ic][:, rc * 128:(rc + 1) * 128], in_=pst[:])
                        if bias_dest is not None:
                            tmp = prep.tile([128, 128], F32, tag="tmpT", name="tmpT")
                            nc.scalar.copy(out=tmp[:], in_=pst[:])
                            nc.tensor.matmul(bps[:], tmp[:], bcol[ic][:, 0:1],
                                             start=(ic == 0), stop=(ic == ichunks - 1))
                    if bias_dest is not None:
                        if bias_extra is not None:
                            nc.vector.tensor_add(out=bias_dest[rc][:], in0=bps[:], in1=bias_extra[rc][:])
                        else:
                            nc.vector.tensor_copy(out=bias_dest[rc][:], in_=bps[:])

            for p in range(2):
                transpose_into(cp_w[p], 4, 2, cpwT[p])
                transpose_into(kv_w[p], 4, 2, kvwT[p])
                transpose_into(ep_w[p], 2, 4, epwT[p])
            transpose_into(ce3_w, 2, 2, ce3wT)

            cpb = [[col(cp_b[p], k * 128, wp, f"cpb{p}{k}") for k in range(4)] for p in range(2)]
            epb = [[col(ep_b[p], k * 128, wp, f"epb{p}{k}") for k in range(2)] for p in range(2)]
            dwb = [col(dw_b, k * 128, wp, f"dwb{k}") for k in range(2)]
            ce3b = [col(ce3_b, k * 128, wp, f"ce3b{k}") for k in range(2)]
            ce1braw = [col(ce1_b, k * 128, wp, f"ce1braw{k}") for k in range(2)]
            # ln gamma cols (scale for res/ce1 wT rows) + ln beta cols (bias fold)
            gcol = [col(ln_g[0], 0, wp, "g0"), col(ln_g[0], 128, wp, "g1"),
                    col(ln_g[2], 0, wp, "g2"), col(ln_g[2], 128, wp, "g3")]
            bcol = [col(ln_g[1], 0, wp, "b0"), col(ln_g[1], 128, wp, "b1"),
                    col(ln_g[3], 0, wp, "b2"), col(ln_g[3], 128, wp, "b3")]
            # bn params as [128, 2] (col j = channels j*128..)
            bnt = {}
            for k in bn:
                t = wp.tile([128, 2], F32, tag=f"t_{k}", name=f"t_{k}")
                nc.scalar.dma_start(out=t[:], in_=bn[k].rearrange("(a p) -> p a", p=128))
                bnt[k] = t
            w9 = []
            for k in range(2):
                t = wp.tile([128, 9], F32, tag=f"w9{k}", name=f"w9{k}")
                nc.scalar.dma_start(out=t[:], in_=dw_w[k * 128:(k + 1) * 128, :])
                w9.append(t)
            diagw = [[wp.tile([128, 128], BF16, tag=f"diag{tap}{k}", name=f"diag{tap}{k}") for k in range(2)] for tap in range(9)]
            for tap in range(9):
                for k in range(2):
                    nc.vector.tensor_scalar_mul(out=diagw[tap][k][:], in0=I_f32[:], scalar1=w9[k][:, tap:tap + 1])

            transpose_into(res_w, 2, 4, reswT, scale_cols=gcol, bias_dest=resb)
            transpose_into(ce1_w, 2, 4, ce1wT, scale_cols=gcol, bias_dest=ce1b, bias_extra=ce1braw)

        # ---------- DRAM scratch ----------
        dram_cm = tc.tile_pool(name="dram", bufs=1, space="DRAM")
        dramp = dram_cm.__enter__()
        e_dram = [[dramp.tile([128, N], BF16, tag=f"e_d{i}{k}", name=f"e_d{i}{k}") for k in range(2)] for i in range(IMG)]
        r_dram = [[dramp.tile([128, N], BF16, tag=f"r_d{i}{k}", name=f"r_d{i}{k}") for k in range(2)] for i in range(IMG)]
        cc_in = dramp.tile([128, 10], F32, tag="cc_in", name="cc_in")
        cc_out = dramp.tile([128, 10], F32, tag="cc_out", name="cc_out")

        # ---------- main pools ----------
        with tc.tile_pool(name="big", bufs=1) as bigp, \
             tc.tile_pool(name="sm2", bufs=2) as sm2, \
             tc.tile_pool(name="sm3", bufs=4) as sm3, \
             tc.tile_pool(name="rch", bufs=3) as rchp, \
             tc.tile_pool(name="tiny", bufs=2) as tinyp, \
             tc.tile_pool(name="ps_big", bufs=5, space="PSUM") as ps_big, \
             tc.tile_pool(name="ps_ctx", bufs=1, space="PSUM") as ps_ctx, \
             tc.tile_pool(name="ps_row", bufs=1, space="PSUM") as ps_row:

            y1u1 = [[bigp.tile([128, N], BF16, tag=f"y1u1_{p}{k}", name=f"y1u1_{p}{k}") for k in range(4)] for p in range(2)]
            o_fm = [[bigp.tile([128, N], BF16, tag=f"o_{p}{k}", name=f"o_{p}{k}") for k in range(2)] for p in range(2)]
            e1 = [bigp.tile([128, N], BF16, tag=f"e1_{k}", name=f"e1_{k}") for k in range(2)]
            bdE = [[bigp.tile([128, 128], BF16, tag=f"bdE{p}{k}", name=f"bdE{p}{k}") for k in range(2)] for p in range(2)]
            r2 = [[bigp.tile([128, 1], F32, tag=f"r2_{p}{k}", name=f"r2_{p}{k}") for k in range(2)] for p in range(2)]

            for img in range(IMG):
                ecol0 = img * NCH

                # ===== paths: cp -> kv/ctx -> softmax =====
                for p in range(2):
                    for nch in range(NCH):
                        nsl = slice(nch * NCK, (nch + 1) * NCK)
                        xc = []
                        for kc in range(2):
                            st = sm2.tile([128, NCK], F32, tag=f"xstage{kc}", name=f"xstage{kc}")
                            nc.sync.dma_start(out=st[:], in_=xd[p][img, kc * 128:(kc + 1) * 128, nsl])
                            xb = sm2.tile([128, NCK], BF16, tag=f"xc{kc}", name=f"xc{kc}")
                            nc.gpsimd.tensor_copy(out=xb[:], in_=st[:])
                            xc.append(xb)
                        for oc in range(4):
                            ps = ps_big.tile([128, NCK], F32, tag="big", name="big")
                            nc.tensor.matmul(ps[:], cpwT[p][0][:, oc * 128:(oc + 1) * 128], xc[0][:],
                                             start=True, stop=False)
                            nc.tensor.matmul(ps[:], cpwT[p][1][:, oc * 128:(oc + 1) * 128], xc[1][:],
                                             start=False, stop=True)
                            if oc % 2 == 0:
                                nc.scalar.activation(out=y1u1[p][oc][:, nsl], in_=ps[:], func=AF.Relu,
                                                     bias=cpb[p][oc][:, 0:1], scale=1.0)
                            else:
                                nc.vector.tensor_scalar(out=y1u1[p][oc][:, nsl], in0=ps[:],
                                                        scalar1=cpb[p][oc][:, 0:1], scalar2=0.0,
                                                        op0=OP.add, op1=OP.max)
                    ctx_ps = ps_ctx.tile([128, 256], F32, tag="ctx", name="ctx")
                    ctxA = ctx_ps[:, 0:128]
                    ctxB = ctx_ps[:, 128:256]
                    for tch in range(32):
                        tsl = slice(tch * 128, (tch + 1) * 128)
                        ps = ps_big.tile([128, 512], F32, tag="big", name="big")
                        nc.tensor.matmul(ps[:], y1u1[p][2][:, tsl], kvwT[p][0][:], start=True, stop=False)
                        nc.tensor.matmul(ps[:], y1u1[p][3][:, tsl], kvwT[p][1][:], start=False, stop=True)
                        kvsb = sm3.tile([128, 512], BF16, tag="kvsb", name="kvsb")
                        if tch % 2 == 0:
                            nc.scalar.copy(out=kvsb[:], in_=ps[:])
                        else:
                            nc.vector.tensor_copy(out=kvsb[:], in_=ps[:])
                        nc.tensor.matmul(ctxA, kvsb[:, 0:128], kvsb[:, 256:384],
                                         start=(tch == 0), stop=(tch == 31))
                        nc.tensor.matmul(ctxB, kvsb[:, 128:256], kvsb[:, 384:512],
                                         start=(tch == 0), stop=(tch == 31))
                    for dc in range(2):
                        nc.gpsimd.memset(bdE[p][dc][:], 0.0)
                    for h in range(8):
                        dc, ro = h // 4, (h % 4) * 32
                        src = ctxA if dc == 0 else ctxB
                        nc.scalar.activation(out=bdE[p][dc][ro:ro + 32, ro:ro + 32],
                                             in_=src[ro:ro + 32, ro:ro + 32], func=AF.Exp, scale=SCALE)
                    for ec in range(2):
                        sps = ps_big.tile([128, 1], F32, tag="big", name="big")
                        nc.tensor.matmul(sps[:], bdE[p][ec][:], ones_col[:], start=True, stop=True)
                        nc.vector.reciprocal(out=r2[p][ec][:], in_=sps[:])

                # ===== ce helpers (emitted inside p==1 ep loop) =====
                e2ch = {}
                r_live = {}

                def emit_res_ce1(j):
                    jsl = slice(j * NCK, (j + 1) * NCK)
                    rhs4 = [o_fm[0][0][:, jsl], o_fm[0][1][:, jsl], o_fm[1][0][:, jsl], o_fm[1][1][:, jsl]]
                    for oc in range(2):
                        ps = ps_big.tile([128, NCK], F32, tag="big", name="big")
                        for kc in range(4):
                            nc.tensor.matmul(ps[:], reswT[kc][:, oc * 128:(oc + 1) * 128], rhs4[kc],
                                             start=(kc == 0), stop=(kc == 3))
                        rc_t = rchp.tile([128, NCK], BF16, tag=f"rch{oc}", name=f"rch{oc}")
                        nc.vector.tensor_scalar(out=rc_t[:], in0=ps[:], scalar1=resb[oc][:, 0:1],
                                                scalar2=0.0, op0=OP.add, op1=OP.add,
                                                accum_out=stats_acc[:, 2, oc, ecol0 + j:ecol0 + j + 1])
                        nc.sync.dma_start(out=r_dram[img][oc][:, jsl], in_=rc_t[:])
                        sq = tinyp.tile([128, NCK], BF16, tag="sqt", name="sqt")
                        nc.scalar.activation(out=sq[:], in_=rc_t[:], func=AF.Square,
                                             accum_out=stats_acc[:, 3, oc, ecol0 + j:ecol0 + j + 1])
                        r_live[(oc, j)] = rc_t
                        ps = ps_big.tile([128, NCK], F32, tag="big", name="big")
                        for kc in range(4):
                            nc.tensor.matmul(ps[:], ce1wT[kc][:, oc * 128:(oc + 1) * 128], rhs4[kc],
                                             start=(kc == 0), stop=(kc == 3))
                        nc.vector.tensor_scalar(out=e1[oc][:, jsl], in0=ps[:], scalar1=ce1b[oc][:, 0:1],
                                                scalar2=0.0, op0=OP.add, op1=OP.add)

                def emit_dw(j):
                    h0 = j * 8
                    taps = [(0, 0)] + [(dy, dx) for dy in (-1, 0, 1) for dx in (-1, 0, 1) if (dy, dx) != (0, 0)]
                    for cc in range(2):
                        ps = ps_big.tile([128, NCK], F32, tag="big", name="big")
                        pv = ps[:].rearrange("p (h w) -> p h w", w=64)
                        e1v = e1[cc][:].rearrange("p (h w) -> p h w", w=64)
                        for idx, (dy, dx) in enumerate(taps):
                            tap = (dy + 1) * 3 + (dx + 1)
                            ws, wc = max(0, -dx), 64 - abs(dx)
                            hs = max(0, -(h0 + dy))
                            he = min(8, 64 - h0 - dy)
                            nc.tensor.matmul(pv[:, hs:he, ws:ws + wc],
                                             diagw[tap][cc][:],
                                             e1v[:, h0 + hs + dy:h0 + he + dy, ws + dx:ws + dx + wc],
                                             start=(idx == 0), stop=(idx == len(taps) - 1))
                        e2c = sm2.tile([128, NCK], BF16, tag=f"e2{cc}", name=f"e2{cc}")
                        nc.scalar.activation(out=e2c[:], in_=ps[:], func=AF.Relu,
                                             bias=dwb[cc][:, 0:1], scale=1.0)
                        e2ch[cc] = e2c

                def emit_ce3(j):
                    jsl = slice(j * NCK, (j + 1) * NCK)
                    for oc in range(2):
                        ps = ps_big.tile([128, NCK], F32, tag="big", name="big")
                        nc.tensor.matmul(ps[:], ce3wT[0][:, oc * 128:(oc + 1) * 128], e2ch[0][:],
                                         start=True, stop=False)
                        nc.tensor.matmul(ps[:], ce3wT[1][:, oc * 128:(oc + 1) * 128], e2ch[1][:],
                                         start=False, stop=True)
                        ec = rchp.tile([128, NCK], BF16, tag=f"ech{oc}", name=f"ech{oc}")
                        nc.scalar.activation(out=ec[:], in_=ps[:], func=AF.Identity,
                                             bias=ce3b[oc][:, 0:1], scale=1.0,
                                             accum_out=stats_acc[:, 0, oc, ecol0 + j:ecol0 + j + 1])
                        nc.sync.dma_start(out=e_dram[img][oc][:, jsl], in_=ec[:])
                        sq = tinyp.tile([128, NCK], BF16, tag="sqt", name="sqt")
                        nc.vector.scalar_tensor_tensor(out=sq[:], in0=ec[:], scalar=1.0, in1=ec[:],
                                                       op0=OP.mult, op1=OP.mult,
                                                       accum_out=stats_acc[:, 1, oc, ecol0 + j:ecol0 + j + 1])
                        ser = tinyp.tile([128, NCK], BF16, tag="ser", name="ser")
                        nc.gpsimd.tensor_add(out=ser[:], in0=ec[:], in1=r_live.pop((oc, j))[:])
                        sq2 = tinyp.tile([128, NCK], BF16, tag="sqt", name="sqt")
                        nc.vector.scalar_tensor_tensor(out=sq2[:], in0=ser[:], scalar=1.0, in1=ser[:],
                                                       op0=OP.mult, op1=OP.mult,
                                                       accum_out=stats_acc[:, 4, oc, ecol0 + j:ecol0 + j + 1])

                # ===== attention out + end-projection + LayerNorm =====
                for nch in range(NCH):
                    for p in range(2):
                        q = 1 - p
                        nsl = slice(nch * NCK, (nch + 1) * NCK)
                        an = []
                        for mc in range(2):
                            ps = ps_big.tile([128, NCK], F32, tag="big", name="big")
                            nc.tensor.matmul(ps[:], bdE[q][mc][:], y1u1[p][2 + mc][:, nsl],
                                             start=True, stop=True)
                            a_t = sm2.tile([128, NCK], BF16, tag=f"a{mc}", name=f"a{mc}")
                            if mc == 0:
                                nc.scalar.activation(out=a_t[:], in_=ps[:], func=AF.Copy,
                                                     scale=r2[q][mc][:, 0:1], bias=0.0)
                            else:
                                nc.vector.tensor_scalar_mul(out=a_t[:], in0=ps[:], scalar1=r2[q][mc][:, 0:1])
                            an.append(a_t)
                        zs = []
                        ps_s = ps_row.tile([1, NCK], F32, tag="rowS", name="rowS")
                        ps_q = ps_row.tile([1, NCK], F32, tag="rowQ", name="rowQ")
                        for cc in range(2):
                            ps = ps_big.tile([128, NCK], F32, tag="big", name="big")
                            nc.tensor.matmul(ps[:], epwT[p][0][:, cc * 128:(cc + 1) * 128],
                                             y1u1[p][0][:, nsl], start=True, stop=False)
                            nc.tensor.matmul(ps[:], epwT[p][1][:, cc * 128:(cc + 1) * 128],
                                             y1u1[p][1][:, nsl], start=False, stop=False)
                            nc.tensor.matmul(ps[:], epwT[p][2][:, cc * 128:(cc + 1) * 128],
                                             an[0][:], start=False, stop=False)
                            nc.tensor.matmul(ps[:], epwT[p][3][:, cc * 128:(cc + 1) * 128],
                                             an[1][:], start=False, stop=True)
                            xr = sm2.tile([128, NCK], F32, tag="resid", name="resid")
                            nc.sync.dma_start(out=xr[:], in_=xd[p][img, cc * 128:(cc + 1) * 128, nsl])
                            z = sm2.tile([128, NCK], BF16, tag=f"z{p}{cc}", name=f"z{p}{cc}")
                            nc.vector.scalar_tensor_tensor(out=z[:], in0=ps[:], scalar=epb[p][cc][:, 0:1],
                                                           in1=xr[:], op0=OP.add, op1=OP.add)
                            zq = sm2.tile([128, NCK], BF16, tag="zq", name="zq")
                            nc.gpsimd.tensor_mul(out=zq[:], in0=z[:], in1=z[:])
                            nc.tensor.matmul(ps_s[:], ones_col[:], z[:], start=(cc == 0), stop=(cc == 1))
                            nc.tensor.matmul(ps_q[:], ones_col[:], zq[:], start=(cc == 0), stop=(cc == 1))
                            zs.append(z)
                        # per-nch LayerNorm row stats: [1,512] -> [128,4]
                        srq = tinyp.tile([33, NCK], F32, tag="srq", name="srq")
                        nc.scalar.copy(out=srq[0:1, :], in_=ps_s[:])
                        nc.vector.tensor_copy(out=srq[32:33, :], in_=ps_q[:])
                        rs = tinyp.tile([128, 8], F32, tag="rs", name="rs")
                        nc.sync.dma_start(out=rs[:, 0:4], in_=srq[0:1, :])
                        nc.sync.dma_start(out=rs[:, 4:8], in_=srq[32:33, :])
                        ms = tinyp.tile([128, 8], F32, tag="ms", name="ms")
                        nc.scalar.mul(out=ms[:], in_=rs[:], mul=1.0 / 256.0)
                        var = tinyp.tile([128, 4], F32, tag="var", name="var")
                        nc.vector.tensor_mul(out=var[:], in0=ms[:, 0:4], in1=ms[:, 0:4])
                        nc.vector.tensor_sub(out=var[:], in0=ms[:, 4:8], in1=var[:])
                        nc.scalar.activation(out=var[:], in_=var[:], func=AF.Sqrt,
                                             bias=eps_col[:, 0:1], scale=1.0)
                        rstd = tinyp.tile([128, 4], F32, tag="rstd", name="rstd")
                        nc.vector.reciprocal(out=rstd[:], in_=var[:])
                        mrstd = tinyp.tile([128, 4], F32, tag="mrstd", name="mrstd")
                        nc.vector.tensor_mul(out=mrstd[:], in0=ms[:, 0:4], in1=rstd[:])
                        rstd_h = tinyp.tile([128, 4], BF16, tag="rstd_h", name="rstd_h")
                        nc.gpsimd.tensor_copy(out=rstd_h[:], in_=rstd[:])
                        mrstd_h = tinyp.tile([128, 4], BF16, tag="mrstd_h", name="mrstd_h")
                        nc.gpsimd.tensor_copy(out=mrstd_h[:], in_=mrstd[:])
                        rrow = tinyp.tile([1, NCK], BF16, tag="rrow", name="rrow")
                        nc.sync.dma_start(out=rrow[:], in_=rstd_h[:])
                        mrow = tinyp.tile([1, NCK], BF16, tag="mrow", name="mrow")
                        nc.sync.dma_start(out=mrow[:], in_=mrstd_h[:])
                        rbc = sm2.tile([128, NCK], BF16, tag="rbc", name="rbc")
                        nc.gpsimd.partition_broadcast(rbc[:], rrow[:])
                        mbc = sm2.tile([128, NCK], BF16, tag="mbc", name="mbc")
                        nc.gpsimd.partition_broadcast(mbc[:], mrow[:])
                        for cc in range(2):
                            tmn = sm2.tile([128, NCK], BF16, tag="tmn", name="tmn")
                            nc.vector.tensor_mul(out=tmn[:], in0=zs[cc][:], in1=rbc[:])
                            nc.gpsimd.tensor_sub(out=o_fm[p][cc][:, nsl], in0=tmn[:], in1=mbc[:])
                        if p == 1:
                            if nch >= 1:
                                emit_res_ce1(nch - 1)
                            if nch >= 2:
                                emit_dw(nch - 2)
                                emit_ce3(nch - 2)
                emit_res_ce1(NCH - 1)
                emit_dw(NCH - 2)
                emit_ce3(NCH - 2)
                emit_dw(NCH - 1)
                emit_ce3(NCH - 1)

            # ===== BatchNorm stats AllReduce =====
            red = sm2.tile([128, 10], F32, tag="red", name="red")
            nc.vector.tensor_reduce(out=red[:].rearrange("p (a b) -> p a b", a=5),
                                    in_=stats_acc[:], axis=mybir.AxisListType.X, op=OP.add)
            nc.sync.dma_start(out=cc_in[:], in_=red[:])
            if collective:
                nc.gpsimd.collective_compute("AllReduce", OP.add,
                                             replica_groups=[list(range(N_CORES))],
                                             ins=[cc_in[:].opt()], outs=[cc_out[:].opt()])
            else:
                nc.sync.dma_start(out=cc_out[:], in_=cc_in[:])
            nc.sync.dma_start(out=glob[:], in_=cc_out[:])

        # ===== BN coefficient math + final affine =====
        with tc.tile_pool(name="coef", bufs=1) as cf, \
             tc.tile_pool(name="fin", bufs=2) as fin, \
             tc.tile_pool(name="fin2", bufs=2) as fin2:
            def ct(tag):
                return cf.tile([128, 2], F32, tag=tag, name=tag)
            Se, Se2, Sr, Sr2, Ss2 = (glob[:, 2 * i:2 * i + 2] for i in range(5))
            g1t, b1t = bnt["bn1_g"][:], bnt["bn1_b"][:]
            g2t, b2t = bnt["bn2_g"][:], bnt["bn2_b"][:]
            m1 = ct("m1"); nc.scalar.mul(out=m1[:], in_=Se, mul=1.0 / NGLOB)
            v1 = ct("v1"); nc.scalar.mul(out=v1[:], in_=Se2, mul=1.0 / NGLOB)
            t0 = ct("t0"); nc.vector.tensor_mul(out=t0[:], in0=m1[:], in1=m1[:])
            nc.vector.tensor_sub(out=v1[:], in0=v1[:], in1=t0[:])
            sd1 = ct("sd1")
            nc.scalar.activation(out=sd1[:], in_=v1[:], func=AF.Sqrt, bias=eps_col[:, 0:1], scale=1.0)
            s1g1 = ct("s1g1"); nc.vector.reciprocal(out=s1g1[:], in_=sd1[:])
            nc.vector.tensor_mul(out=s1g1[:], in0=s1g1[:], in1=g1t)
            mr = ct("mr"); nc.scalar.mul(out=mr[:], in_=Sr, mul=1.0 / NGLOB)
            m2 = ct("m2"); nc.vector.tensor_add(out=m2[:], in0=mr[:], in1=b1t)
            eer = ct("eer"); nc.vector.tensor_sub(out=eer[:], in0=Ss2, in1=Se2)
            nc.vector.tensor_sub(out=eer[:], in0=eer[:], in1=Sr2)
            nc.scalar.mul(out=eer[:], in_=eer[:], mul=0.5 / NGLOB)
            nc.vector.tensor_mul(out=t0[:], in0=m1[:], in1=mr[:])
            nc.vector.tensor_sub(out=eer[:], in0=eer[:], in1=t0[:])
            nc.vector.tensor_mul(out=eer[:], in0=eer[:], in1=s1g1[:])
            nc.vector.tensor_mul(out=t0[:], in0=b1t, in1=mr[:])
            cross = ct("cross"); nc.vector.tensor_add(out=cross[:], in0=eer[:], in1=t0[:])
            ez2 = ct("ez2"); nc.scalar.mul(out=ez2[:], in_=Sr2, mul=1.0 / NGLOB)
            nc.scalar.mul(out=cross[:], in_=cross[:], mul=2.0)
            nc.vector.tensor_add(out=ez2[:], in0=ez2[:], in1=cross[:])
            nc.vector.tensor_mul(out=t0[:], in0=s1g1[:], in1=s1g1[:])
            nc.vector.tensor_mul(out=t0[:], in0=t0[:], in1=v1[:])
            nc.vector.tensor_add(out=ez2[:], in0=ez2[:], in1=t0[:])
            nc.vector.tensor_mul(out=t0[:], in0=b1t, in1=b1t)
            nc.vector.tensor_add(out=ez2[:], in0=ez2[:], in1=t0[:])
            nc.vector.tensor_mul(out=t0[:], in0=m2[:], in1=m2[:])
            nc.vector.tensor_sub(out=ez2[:], in0=ez2[:], in1=t0[:])
            nc.scalar.activation(out=ez2[:], in_=ez2[:], func=AF.Sqrt, bias=eps_col[:, 0:1], scale=1.0)
            A = ct("A"); nc.vector.reciprocal(out=A[:], in_=ez2[:])
            nc.vector.tensor_mul(out=A[:], in0=A[:], in1=g2t)
            Bc = ct("Bc"); nc.vector.tensor_mul(out=Bc[:], in0=A[:], in1=s1g1[:])
            Cc = ct("Cc"); nc.vector.tensor_mul(out=Cc[:], in0=s1g1[:], in1=m1[:])
            nc.vector.tensor_sub(out=Cc[:], in0=b1t, in1=Cc[:])
            nc.vector.tensor_sub(out=Cc[:], in0=Cc[:], in1=m2[:])
            nc.vector.tensor_mul(out=Cc[:], in0=Cc[:], in1=A[:])
            nc.vector.tensor_add(out=Cc[:], in0=Cc[:], in1=b2t)

            for img in range(IMG):
                for cc in range(2):
                    e_rb = fin.tile([128, N], BF16, tag="e_rb", name="e_rb")
                    nc.sync.dma_start(out=e_rb[:], in_=e_dram[img][cc][:])
                    r_rb = fin.tile([128, N], BF16, tag="r_rb", name="r_rb")
                    nc.scalar.dma_start(out=r_rb[:], in_=r_dram[img][cc][:])
                    ot = fin.tile([128, N], F32, tag="ot", name="ot")
                    for nch in range(NCH):
                        nsl = slice(nch * NCK, (nch + 1) * NCK)
                        ebc = fin2.tile([128, NCK], F32, tag="ebc", name="ebc")
                        nc.gpsimd.tensor_scalar(out=ebc[:], in0=e_rb[:, nsl], scalar1=Bc[:, cc:cc + 1],
                                                scalar2=Cc[:, cc:cc + 1], op0=OP.mult, op1=OP.add)
                        nc.vector.scalar_tensor_tensor(out=ot[:, nsl], in0=r_rb[:, nsl], scalar=A[:, cc:cc + 1],
                                                       in1=ebc[:], op0=OP.mult, op1=OP.add)
                    nc.sync.dma_start(out=y[img, cc * 128:(cc + 1) * 128, :], in_=ot[:])

        dram_cm.__exit__(None, None, None)
        wpool.__exit__(None, None, None)

    nc.finalize()
    return nc


_CACHE = {}


def _get_nc():
    if "nc" not in _CACHE:
        _CACHE["nc"] = build_nc()
    return _CACHE["nc"]


def kernel(trace=False, **inputs):
    nc = _get_nc()
    f = lambda k: np.ascontiguousarray(np.asarray(inputs[k]), dtype=np.float32)
    x1 = f("x1").reshape(16, C, N)
    x2 = f("x2").reshape(16, C, N)
    common = {}
    for k in ("cp1_w", "cp1_b", "cp2_w", "cp2_b", "kv1_w", "kv2_w",
              "ep1_w", "ep1_b", "ep2_w", "ep2_b", "ln1_g", "ln1_b", "ln2_g", "ln2_b",
              "res_w", "ce1_w", "ce1_b", "dw_b", "ce3_w", "ce3_b",
              "bn1_g", "bn1_b", "bn2_g", "bn2_b"):
        common[k] = f(k)
    common["dw_w"] = f("dw_w").reshape(C, 9)
    in_maps = []
    for c in range(N_CORES):
        m = dict(common)
        m["x1"] = x1[IMG * c:IMG * (c + 1)]
        m["x2"] = x2[IMG * c:IMG * (c + 1)]
        in_maps.append(m)
    res = run_bass_kernel_spmd(nc, in_maps, core_ids=list(range(N_CORES)), trace=trace)
    out = np.concatenate([res.results[c]["y"].reshape(IMG, C, 64, 64) for c in range(N_CORES)], axis=0)
    if trace:
        kernel.last_exec_time_ns = res.exec_time_ns
    return out.astype(np.float32)
